# revision 31
# baseline (speedup 1.0000x reference)
"""Trainium2 Bass kernel for the BNN MC-sample MLP (nn_BNN_36532991820264).

Math (per MC sample s):
    W_l = we_l[s] * exp(0.5*wv_l) + wm_l     (sampled weights)
    b_l = be_l[s] * exp(0.5*bv_l) + bm_l     (sampled biases)
    act = relu(act @ W0 + b0); act = relu(act @ W1 + b1); out = act @ Wl + bl

Sharding: 32 MC samples -> 4 per core x 8 cores (embarrassingly parallel).
Per-core layout is feature-major: activations live in SBUF as actT[k_slab]
(feature partition, batch free), so each layer's PSUM output (o_tile, b_tile)
is directly the next layer's input slab -- no transposes anywhere.

Matmul: out(o,b) += W[k,o]^T @ actT[k,b] with W stationary (128x128 fp16),
actT moving (128x512 fp16), fp32 PSUM accumulation over k.
"""

import os

os.environ.setdefault("JAX_PLATFORMS", "axon")

from contextlib import ExitStack

import numpy as np

import concourse.bass as bass
import concourse.mybir as mybir
import concourse.tile as tile
from concourse import bacc
from concourse.bass_utils import run_bass_kernel_spmd

S, B, D_IN, D_H, D_OUT = 32, 1024, 784, 1024, 10
NCORES = 8
SPC = S // NCORES  # samples per core

F16 = mybir.dt.float16
F32 = mybir.dt.float32
AF = mybir.ActivationFunctionType

# k-slabs: [offset, size] chunks of <=128 along the contraction dim
K0 = [(i * 128, min(128, D_IN - i * 128)) for i in range((D_IN + 127) // 128)]
K1 = [(i * 128, 128) for i in range(D_H // 128)]
NO = D_H // 128  # output tiles of 128 (layers 0/1)
NB = B // 512    # batch tiles of 512

_CACHE = {}


def _build_program(repeat=1, pair_b=True):
    nc = bacc.Bacc(
        "TRN2",
        target_bir_lowering=False,
        debug=False,
        enable_asserts=False,
        num_devices=NCORES,
    )

    # ---- per-core DRAM tensors (shard shapes) ----
    xT_d = nc.dram_tensor("xT", [D_IN, B], F16, kind="ExternalInput").ap()
    wm0_d = nc.dram_tensor("wm0", [D_IN, D_H], F16, kind="ExternalInput").ap()
    wv0_d = nc.dram_tensor("wv0", [D_IN, D_H], F16, kind="ExternalInput").ap()
    wm1_d = nc.dram_tensor("wm1", [D_H, D_H], F16, kind="ExternalInput").ap()
    wv1_d = nc.dram_tensor("wv1", [D_H, D_H], F16, kind="ExternalInput").ap()
    wlm_d = nc.dram_tensor("wlm", [D_H, D_OUT], F16, kind="ExternalInput").ap()
    wlv_d = nc.dram_tensor("wlv", [D_H, D_OUT], F16, kind="ExternalInput").ap()
    # biases pre-laid-out on host as (128, 8) / (10, 1)
    bm0_d = nc.dram_tensor("bm0", [128, NO], F16, kind="ExternalInput").ap()
    bv0_d = nc.dram_tensor("bv0", [128, NO], F16, kind="ExternalInput").ap()
    bm1_d = nc.dram_tensor("bm1", [128, NO], F16, kind="ExternalInput").ap()
    bv1_d = nc.dram_tensor("bv1", [128, NO], F16, kind="ExternalInput").ap()
    blm_d = nc.dram_tensor("blm", [D_OUT, 1], F32, kind="ExternalInput").ap()
    blv_d = nc.dram_tensor("blv", [D_OUT, 1], F32, kind="ExternalInput").ap()
    we0_d = nc.dram_tensor("we0", [SPC, D_IN, D_H], F16, kind="ExternalInput").ap()
    we1_d = nc.dram_tensor("we1", [SPC, D_H, D_H], F16, kind="ExternalInput").ap()
    wel_d = nc.dram_tensor("wel", [SPC, D_H, D_OUT], F16, kind="ExternalInput").ap()
    be0_d = nc.dram_tensor("be0", [SPC, 128, NO], F16, kind="ExternalInput").ap()
    be1_d = nc.dram_tensor("be1", [SPC, 128, NO], F16, kind="ExternalInput").ap()
    bel_d = nc.dram_tensor("bel", [SPC, D_OUT, 1], F32, kind="ExternalInput").ap()
    out_d = nc.dram_tensor("out", [SPC, D_OUT, B], F32, kind="ExternalOutput").ap()

    with tile.TileContext(nc) as tc, ExitStack() as ctx:
        const = ctx.enter_context(tc.tile_pool(name="const", bufs=1))
        stream = ctx.enter_context(tc.tile_pool(name="stream", bufs=2))
        acts = ctx.enter_context(tc.tile_pool(name="acts", bufs=1))
        pss = ctx.enter_context(tc.tile_pool(name="ps", bufs=8, space="PSUM"))
        outp = ctx.enter_context(tc.tile_pool(name="outp", bufs=2))

        # ---------- PE warm-up ----------
        # ~4us of dummy matmuls overlapping the DMA prologue so the HAM clock
        # gate releases (1.2 -> 2.4 GHz) before the first real matmul issues.
        warm = stream.tile([128, 64], F16, name="warm", tag="warm")
        nc.vector.memset(warm[:], 0.0)
        wps = pss.tile([64, 64], F32, name="warm_ps", tag="ps")
        for _ in range(48):
            nc.tensor.matmul(wps[:], warm[:], warm[:], start=True, stop=True)

        # ---------- one-time setup, interleaved with sample-0 prep ----------
        # Emission order IS DMA-queue order: stream each layer's params slab
        # by slab together with sample 0's noise so the PE can start layer 0
        # after ~one slab instead of after the full 9MB parameter load.
        xT = []

        def setup_layer(wm_d, wv_d, we_d, K, O, lname, with_x=False):
            sig, m, W = [], [], []
            for i, (off, sz) in enumerate(K):
                vt = stream.tile([sz, O], F16, name=f"vt{lname}_{i}", tag=f"we{lname}_{i}")
                nc.sync.dma_start(vt[:], wv_d[off : off + sz, :])
                mt = const.tile([sz, O], F16, name=f"m{lname}_{i}", tag=f"m{lname}_{i}")
                nc.sync.dma_start(mt[:], wm_d[off : off + sz, :])
                if with_x:
                    xt = const.tile([sz, B], F16, name=f"xT_{i}", tag=f"xT_{i}")
                    nc.sync.dma_start(xt[:], xT_d[off : off + sz, :])
                    xT.append(xt)
                st = const.tile([sz, O], F16, name=f"sig{lname}_{i}", tag=f"sig{lname}_{i}")
                nc.scalar.activation(st[:], vt[:], AF.Exp, scale=0.5)
                # sample 0's noise slab + sampling, right behind the params
                wt = stream.tile([sz, O], F16, name=f"we{lname}_{i}_u0", tag=f"we{lname}_{i}")
                nc.sync.dma_start(wt[:], we_d[0, off : off + sz, :])
                nc.vector.tensor_mul(wt[:], wt[:], st[:])
                nc.vector.tensor_add(wt[:], wt[:], mt[:])
                sig.append(st)
                m.append(mt)
                W.append(wt)
            return sig, m, W

        def setup_bias(bm_d, bv_d, be_d, P, W_, lname, dt):
            bmt = const.tile([P, W_], dt, name=f"bm{lname}", tag=f"bm{lname}")
            nc.sync.dma_start(bmt[:], bm_d[:, :])
            bvt = stream.tile([P, W_], dt, name=f"bv{lname}", tag=f"be{lname}")
            nc.sync.dma_start(bvt[:], bv_d[:, :])
            bst = const.tile([P, W_], dt, name=f"bsig{lname}", tag=f"bsig{lname}")
            nc.scalar.activation(bst[:], bvt[:], AF.Exp, scale=0.5)
            bt = stream.tile([P, W_], dt, name=f"b{lname}_u0", tag=f"be{lname}")
            nc.sync.dma_start(bt[:], be_d[0, :, :])
            nc.vector.tensor_mul(bt[:], bt[:], bst[:])
            nc.vector.tensor_add(bt[:], bt[:], bmt[:])
            return bst, bmt, bt

        sig0, m0, W0_0 = setup_layer(wm0_d, wv0_d, we0_d, K0, D_H, "0", with_x=True)
        bsig0, bm0t, b0_0 = setup_bias(bm0_d, bv0_d, be0_d, 128, NO, "0", F16)
        sig1, m1, W1_0 = setup_layer(wm1_d, wv1_d, we1_d, K1, D_H, "1")
        bsig1, bm1t, b1_0 = setup_bias(bm1_d, bv1_d, be1_d, 128, NO, "1", F16)
        sigl, ml, Wl_0 = setup_layer(wlm_d, wlv_d, wel_d, K1, D_OUT, "l")
        bsigl, blmt, bl_0 = setup_bias(blm_d, blv_d, bel_d, D_OUT, 1, "l", F32)

        # ---------- per-sample weight/bias sampling (DMA + DVE) ----------
        def emit_prep(s, u):
            def prep_w(we_d, K, O, sig, m, lname):
                W = []
                for i, (off, sz) in enumerate(K):
                    wt = stream.tile(
                        [sz, O], F16, name=f"we{lname}_{i}_u{u}", tag=f"we{lname}_{i}"
                    )
                    nc.sync.dma_start(wt[:], we_d[s, off : off + sz, :])
                    nc.vector.tensor_mul(wt[:], wt[:], sig[i][:])
                    nc.vector.tensor_add(wt[:], wt[:], m[i][:])
                    W.append(wt)
                return W

            def prep_b(be_d, P, W_, bsig, bmt, lname, dt=F16):
                bt = stream.tile([P, W_], dt, name=f"b{lname}_u{u}", tag=f"be{lname}")
                nc.sync.dma_start(bt[:], be_d[s, :, :])
                nc.vector.tensor_mul(bt[:], bt[:], bsig[:])
                nc.vector.tensor_add(bt[:], bt[:], bmt[:])
                return bt

            return dict(
                W0=prep_w(we0_d, K0, D_H, sig0, m0, "0"),
                W1=prep_w(we1_d, K1, D_H, sig1, m1, "1"),
                Wl=prep_w(wel_d, K1, D_OUT, sigl, ml, "l"),
                b0=prep_b(be0_d, 128, NO, bsig0, bm0t, "0"),
                b1=prep_b(be1_d, 128, NO, bsig1, bm1t, "1"),
                bl=prep_b(bel_d, D_OUT, 1, bsigl, blmt, "l", F32),
            )

        # ---------- layer emission: matmul groups + ACT evacuation ----------
        def emit_layer(u, pname, W, rhs, K, act_out, func, bias, odt):
            """out[o, b] = func(sum_k W[k][:,o].T @ rhs[k][:,b] + bias[:,o])."""
            if pair_b:
                # both b-tiles per (o,k): consecutive matmuls share lhsT so the
                # PE can keep the stationary operand across the pair.
                for o in range(NO):
                    pss_b = [
                        pss.tile([128, 512], F32, name=f"{pname}_{o}_{b}_u{u}", tag="ps")
                        for b in range(NB)
                    ]
                    for i in range(len(K)):
                        for b in range(NB):
                            nc.tensor.matmul(
                                pss_b[b][:],
                                W[i][:, o * 128 : (o + 1) * 128],
                                rhs[i][:, b * 512 : (b + 1) * 512],
                                start=(i == 0),
                                stop=(i == len(K) - 1),
                            )
                    for b in range(NB):
                        nc.scalar.activation(
                            act_out[o][:, b * 512 : (b + 1) * 512],
                            pss_b[b][:],
                            func,
                            bias=bias[:, o : o + 1],
                        )
            else:
                for o in range(NO):
                    for b in range(NB):
                        ps = pss.tile(
                            [128, 512], F32, name=f"{pname}_{o}_{b}_u{u}", tag="ps"
                        )
                        for i in range(len(K)):
                            nc.tensor.matmul(
                                ps[:],
                                W[i][:, o * 128 : (o + 1) * 128],
                                rhs[i][:, b * 512 : (b + 1) * 512],
                                start=(i == 0),
                                stop=(i == len(K) - 1),
                            )
                        nc.scalar.activation(
                            act_out[o][:, b * 512 : (b + 1) * 512],
                            ps[:],
                            func,
                            bias=bias[:, o : o + 1],
                        )

        # ---------- per-sample compute (PE + ACT evac) ----------
        def emit_compute(s, u, P):
            act0 = [
                acts.tile([128, B], F16, name=f"act0_{o}_u{u}", tag=f"act0_{o}")
                for o in range(NO)
            ]
            emit_layer(
                u, "ps0", P["W0"], xT, K0, act0, AF.Relu, P["b0"], F16
            )

            act1 = [
                acts.tile([128, B], F16, name=f"act1_{o}_u{u}", tag=f"act1_{o}")
                for o in range(NO)
            ]
            emit_layer(
                u, "ps1", P["W1"], act0, K1, act1, AF.Relu, P["b1"], F16
            )

            out_t = outp.tile([16, B], F32, name=f"out_u{u}", tag="out")
            for b in range(NB):
                ps = pss.tile([128, 512], F32, name=f"psl_{b}_u{u}", tag="ps")
                for i in range(len(K1)):
                    nc.tensor.matmul(
                        ps[:D_OUT, :],
                        P["Wl"][i][:, :],
                        act1[i][:, b * 512 : (b + 1) * 512],
                        start=(i == 0),
                        stop=(i == len(K1) - 1),
                    )
                nc.vector.tensor_scalar_add(
                    out_t[:D_OUT, b * 512 : (b + 1) * 512],
                    ps[:D_OUT, :],
                    P["bl"][:, 0:1],
                )
            nc.sync.dma_start(out_d[s, :, :], out_t[:D_OUT, :])

        # software-pipeline: emit sample s+1's prep before sample s's compute
        # so the DVE queue stays ahead of the PE queue. `repeat` re-runs the
        # whole per-sample sequence (timing-only variants). Sample 0's prep
        # was emitted inside the setup interleave above.
        seq = [s for _ in range(repeat) for s in range(SPC)]
        assert seq[0] == 0
        preps = dict(W0=W0_0, W1=W1_0, Wl=Wl_0, b0=b0_0, b1=b1_0, bl=bl_0)
        for u, s in enumerate(seq):
            nxt = emit_prep(seq[u + 1], u + 1) if u + 1 < len(seq) else None
            emit_compute(s, u, preps)
            preps = nxt

    nc.compile()
    return nc


def _f16(a):
    return np.ascontiguousarray(np.asarray(a), dtype=np.float16)


def _bias_128(b):
    # (1024,) -> (128, 8): feature o = t*128 + p lands at [p, t]
    return np.ascontiguousarray(np.asarray(b, np.float16).reshape(NO, 128).T)


def _shard_inputs(inputs):
    x = np.asarray(inputs["x"])
    xT = _f16(x.T)
    shared = dict(
        xT=xT,
        wm0=_f16(inputs["wm0"]),
        wv0=_f16(inputs["wv0"]),
        wm1=_f16(inputs["wm1"]),
        wv1=_f16(inputs["wv1"]),
        wlm=_f16(inputs["wlm"]),
        wlv=_f16(inputs["wlv"]),
        bm0=_bias_128(inputs["bm0"]),
        bv0=_bias_128(inputs["bv0"]),
        bm1=_bias_128(inputs["bm1"]),
        bv1=_bias_128(inputs["bv1"]),
        blm=np.ascontiguousarray(
            np.asarray(inputs["blm"], np.float32).reshape(D_OUT, 1)
        ),
        blv=np.ascontiguousarray(
            np.asarray(inputs["blv"], np.float32).reshape(D_OUT, 1)
        ),
    )
    we0 = np.asarray(inputs["we0"])
    we1 = np.asarray(inputs["we1"])
    wel = np.asarray(inputs["wel"])
    be0 = np.asarray(inputs["be0"]).reshape(S, D_H)
    be1 = np.asarray(inputs["be1"]).reshape(S, D_H)
    bel = np.asarray(inputs["bel"]).reshape(S, D_OUT)

    in_maps = []
    for c in range(NCORES):
        sl = slice(c * SPC, (c + 1) * SPC)
        m = dict(shared)
        m["we0"] = _f16(we0[sl])
        m["we1"] = _f16(we1[sl])
        m["wel"] = _f16(wel[sl])
        m["be0"] = np.ascontiguousarray(
            be0[sl].astype(np.float16).reshape(SPC, NO, 128).transpose(0, 2, 1)
        )
        m["be1"] = np.ascontiguousarray(
            be1[sl].astype(np.float16).reshape(SPC, NO, 128).transpose(0, 2, 1)
        )
        m["bel"] = np.ascontiguousarray(
            bel[sl].astype(np.float32).reshape(SPC, D_OUT, 1)
        )
        in_maps.append(m)
    return in_maps


def _get_program(repeat=1, pair_b=True):
    key = (repeat, pair_b)
    if key not in _CACHE:
        _CACHE[key] = _build_program(repeat, pair_b)
    return _CACHE[key]


def run(inputs, trace=False, repeat=1, pair_b=True, **kw):
    nc = _get_program(repeat, pair_b)
    in_maps = _shard_inputs(inputs)
    try:
        res = run_bass_kernel_spmd(
            nc, in_maps, core_ids=list(range(NCORES)), trace=trace, **kw
        )
    except ModuleNotFoundError:
        # this container lacks the axon NTFF profile hook; rerun untraced
        os.environ["BASS_NEVER_TRACE"] = "1"
        res = run_bass_kernel_spmd(
            nc, in_maps, core_ids=list(range(NCORES)), trace=False, **kw
        )
    # per-core out: (SPC, 10, B) -> full (S, B, 10)
    parts = [
        np.transpose(res.results[c]["out"], (0, 2, 1)) for c in range(NCORES)
    ]
    full = np.concatenate(parts, axis=0).astype(np.float32)
    return full, res


def kernel(**inputs) -> np.ndarray:
    out, _ = run(inputs, trace=False)
    return out


# revision 43
# speedup vs baseline: 1.0687x; 1.0687x over previous
"""Trainium2 Bass kernel for the BNN MC-sample MLP (nn_BNN_36532991820264).

Math (per MC sample s):
    W_l = we_l[s] * exp(0.5*wv_l) + wm_l     (sampled weights)
    b_l = be_l[s] * exp(0.5*bv_l) + bm_l     (sampled biases)
    act = relu(act @ W0 + b0); act = relu(act @ W1 + b1); out = act @ Wl + bl

Sharding: 32 MC samples -> 4 per core x 8 cores (embarrassingly parallel).
Per-core layout is feature-major: activations live in SBUF as actT[k_slab]
(feature partition, batch free), so each layer's PSUM output (o_tile, b_tile)
is directly the next layer's input slab -- no transposes anywhere.

Matmul: out(o,b) += W[k,o]^T @ actT[k,b] with W stationary (128x128 fp16),
actT moving (128x512 fp16), fp32 PSUM accumulation over k.
"""

import os

os.environ.setdefault("JAX_PLATFORMS", "axon")

from contextlib import ExitStack

import numpy as np

import concourse.bass as bass
import concourse.mybir as mybir
import concourse.tile as tile
from concourse import bacc
from concourse.bass_utils import run_bass_kernel_spmd

S, B, D_IN, D_H, D_OUT = 32, 1024, 784, 1024, 10
NCORES = 8
SPC = S // NCORES  # samples per core

F16 = mybir.dt.float16
F32 = mybir.dt.float32
AF = mybir.ActivationFunctionType

# k-slabs: [offset, size] chunks of <=128 along the contraction dim
K0 = [(i * 128, min(128, D_IN - i * 128)) for i in range((D_IN + 127) // 128)]
K1 = [(i * 128, 128) for i in range(D_H // 128)]
NO = D_H // 128  # output tiles of 128 (layers 0/1)
NB = B // 512    # batch tiles of 512

_CACHE = {}


def _build_program(repeat=1, pair_b=True, evac_split=True, acts_bufs=2):
    nc = bacc.Bacc(
        "TRN2",
        target_bir_lowering=False,
        debug=False,
        enable_asserts=False,
        num_devices=NCORES,
    )

    # ---- per-core DRAM tensors (shard shapes) ----
    xT_d = nc.dram_tensor("xT", [D_IN, B], F16, kind="ExternalInput").ap()
    wm0_d = nc.dram_tensor("wm0", [D_IN, D_H], F16, kind="ExternalInput").ap()
    wv0_d = nc.dram_tensor("wv0", [D_IN, D_H], F16, kind="ExternalInput").ap()
    wm1_d = nc.dram_tensor("wm1", [D_H, D_H], F16, kind="ExternalInput").ap()
    wv1_d = nc.dram_tensor("wv1", [D_H, D_H], F16, kind="ExternalInput").ap()
    wlm_d = nc.dram_tensor("wlm", [D_H, D_OUT], F16, kind="ExternalInput").ap()
    wlv_d = nc.dram_tensor("wlv", [D_H, D_OUT], F16, kind="ExternalInput").ap()
    # biases pre-laid-out on host as (128, 8) / (10, 1)
    bm0_d = nc.dram_tensor("bm0", [128, NO], F32, kind="ExternalInput").ap()
    bv0_d = nc.dram_tensor("bv0", [128, NO], F32, kind="ExternalInput").ap()
    bm1_d = nc.dram_tensor("bm1", [128, NO], F32, kind="ExternalInput").ap()
    bv1_d = nc.dram_tensor("bv1", [128, NO], F32, kind="ExternalInput").ap()
    blm_d = nc.dram_tensor("blm", [D_OUT, 1], F32, kind="ExternalInput").ap()
    blv_d = nc.dram_tensor("blv", [D_OUT, 1], F32, kind="ExternalInput").ap()
    we0_d = nc.dram_tensor("we0", [SPC, D_IN, D_H], F16, kind="ExternalInput").ap()
    we1_d = nc.dram_tensor("we1", [SPC, D_H, D_H], F16, kind="ExternalInput").ap()
    wel_d = nc.dram_tensor("wel", [SPC, D_H, D_OUT], F16, kind="ExternalInput").ap()
    be0_d = nc.dram_tensor("be0", [SPC, 128, NO], F32, kind="ExternalInput").ap()
    be1_d = nc.dram_tensor("be1", [SPC, 128, NO], F32, kind="ExternalInput").ap()
    bel_d = nc.dram_tensor("bel", [SPC, D_OUT, 1], F32, kind="ExternalInput").ap()
    out_d = nc.dram_tensor("out", [SPC, D_OUT, B], F32, kind="ExternalOutput").ap()

    with tile.TileContext(nc) as tc, ExitStack() as ctx:
        const = ctx.enter_context(tc.tile_pool(name="const", bufs=1))
        stream = ctx.enter_context(tc.tile_pool(name="stream", bufs=2))
        acts = ctx.enter_context(tc.tile_pool(name="acts", bufs=acts_bufs))
        pss = ctx.enter_context(tc.tile_pool(name="ps", bufs=8, space="PSUM"))
        outp = ctx.enter_context(tc.tile_pool(name="outp", bufs=2))

        # ---------- PE warm-up ----------
        # ~4us of dummy matmuls overlapping the DMA prologue so the HAM clock
        # gate releases (1.2 -> 2.4 GHz) before the first real matmul issues.
        warm = stream.tile([128, 64], F16, name="warm", tag="warm")
        nc.vector.memset(warm[:], 0.0)
        wps = pss.tile([64, 64], F32, name="warm_ps", tag="ps")
        for _ in range(48):
            nc.tensor.matmul(wps[:], warm[:], warm[:], start=True, stop=True)

        # ---------- one-time setup, interleaved with sample-0 prep ----------
        # Emission order IS DMA-queue order: stream each layer's params slab
        # by slab together with sample 0's noise so the PE can start layer 0
        # after ~one slab instead of after the full 9MB parameter load.
        xT = []

        def setup_layer(wm_d, wv_d, we_d, K, O, lname, with_x=False):
            sig, m, W = [], [], []
            for i, (off, sz) in enumerate(K):
                vt = stream.tile([sz, O], F16, name=f"vt{lname}_{i}", tag=f"we{lname}_{i}")
                nc.sync.dma_start(vt[:], wv_d[off : off + sz, :])
                mt = const.tile([sz, O], F16, name=f"m{lname}_{i}", tag=f"m{lname}_{i}")
                nc.sync.dma_start(mt[:], wm_d[off : off + sz, :])
                if with_x:
                    xt = const.tile([sz, B], F16, name=f"xT_{i}", tag=f"xT_{i}")
                    nc.sync.dma_start(xt[:], xT_d[off : off + sz, :])
                    xT.append(xt)
                st = const.tile([sz, O], F16, name=f"sig{lname}_{i}", tag=f"sig{lname}_{i}")
                nc.scalar.activation(st[:], vt[:], AF.Exp, scale=0.5)
                # sample 0's noise slab + sampling, right behind the params
                wt = stream.tile([sz, O], F16, name=f"we{lname}_{i}_u0", tag=f"we{lname}_{i}")
                nc.sync.dma_start(wt[:], we_d[0, off : off + sz, :])
                nc.vector.tensor_mul(wt[:], wt[:], st[:])
                nc.vector.tensor_add(wt[:], wt[:], mt[:])
                sig.append(st)
                m.append(mt)
                W.append(wt)
            return sig, m, W

        def setup_bias(bm_d, bv_d, be_d, P, W_, lname, dt):
            bmt = const.tile([P, W_], dt, name=f"bm{lname}", tag=f"bm{lname}")
            nc.sync.dma_start(bmt[:], bm_d[:, :])
            bvt = stream.tile([P, W_], dt, name=f"bv{lname}", tag=f"be{lname}")
            nc.sync.dma_start(bvt[:], bv_d[:, :])
            bst = const.tile([P, W_], dt, name=f"bsig{lname}", tag=f"bsig{lname}")
            nc.scalar.activation(bst[:], bvt[:], AF.Exp, scale=0.5)
            bt = stream.tile([P, W_], dt, name=f"b{lname}_u0", tag=f"be{lname}")
            nc.sync.dma_start(bt[:], be_d[0, :, :])
            nc.vector.tensor_mul(bt[:], bt[:], bst[:])
            nc.vector.tensor_add(bt[:], bt[:], bmt[:])
            return bst, bmt, bt

        sig0, m0, W0_0 = setup_layer(wm0_d, wv0_d, we0_d, K0, D_H, "0", with_x=True)
        bsig0, bm0t, b0_0 = setup_bias(bm0_d, bv0_d, be0_d, 128, NO, "0", F32)
        sig1, m1, W1_0 = setup_layer(wm1_d, wv1_d, we1_d, K1, D_H, "1")
        bsig1, bm1t, b1_0 = setup_bias(bm1_d, bv1_d, be1_d, 128, NO, "1", F32)
        sigl, ml, Wl_0 = setup_layer(wlm_d, wlv_d, wel_d, K1, D_OUT, "l")
        bsigl, blmt, bl_0 = setup_bias(blm_d, blv_d, bel_d, D_OUT, 1, "l", F32)

        # ---------- per-sample weight/bias sampling (DMA + DVE) ----------
        def emit_prep(s, u):
            def prep_w(we_d, K, O, sig, m, lname):
                W = []
                for i, (off, sz) in enumerate(K):
                    wt = stream.tile(
                        [sz, O], F16, name=f"we{lname}_{i}_u{u}", tag=f"we{lname}_{i}"
                    )
                    nc.sync.dma_start(wt[:], we_d[s, off : off + sz, :])
                    nc.vector.tensor_mul(wt[:], wt[:], sig[i][:])
                    nc.vector.tensor_add(wt[:], wt[:], m[i][:])
                    W.append(wt)
                return W

            def prep_b(be_d, P, W_, bsig, bmt, lname, dt=F16):
                bt = stream.tile([P, W_], dt, name=f"b{lname}_u{u}", tag=f"be{lname}")
                nc.sync.dma_start(bt[:], be_d[s, :, :])
                nc.vector.tensor_mul(bt[:], bt[:], bsig[:])
                nc.vector.tensor_add(bt[:], bt[:], bmt[:])
                return bt

            return dict(
                W0=prep_w(we0_d, K0, D_H, sig0, m0, "0"),
                W1=prep_w(we1_d, K1, D_H, sig1, m1, "1"),
                Wl=prep_w(wel_d, K1, D_OUT, sigl, ml, "l"),
                b0=prep_b(be0_d, 128, NO, bsig0, bm0t, "0", F32),
                b1=prep_b(be1_d, 128, NO, bsig1, bm1t, "1", F32),
                bl=prep_b(bel_d, D_OUT, 1, bsigl, blmt, "l", F32),
            )

        # ---------- layer emission: matmul groups + ACT evacuation ----------
        def emit_layer(u, pname, W, rhs, K, act_out, func, bias, odt):
            """out[o, b] = func(sum_k W[k][:,o].T @ rhs[k][:,b] + bias[:,o])."""
            if pair_b:
                # both b-tiles per (o,k): consecutive matmuls share lhsT so the
                # PE can keep the stationary operand across the pair.
                for o in range(NO):
                    pss_b = [
                        pss.tile([128, 512], F32, name=f"{pname}_{o}_{b}_u{u}", tag="ps")
                        for b in range(NB)
                    ]
                    for i in range(len(K)):
                        for b in range(NB):
                            nc.tensor.matmul(
                                pss_b[b][:],
                                W[i][:, o * 128 : (o + 1) * 128],
                                rhs[i][:, b * 512 : (b + 1) * 512],
                                start=(i == 0),
                                stop=(i == len(K) - 1),
                            )
                    for b in range(NB):
                        if evac_split and b % 2 == 1:
                            # relu(psum + bias) on the DVE to halve the
                            # ScalarE evacuation queue depth
                            nc.vector.tensor_scalar(
                                act_out[o][:, b * 512 : (b + 1) * 512],
                                pss_b[b][:],
                                bias[:, o : o + 1],
                                0.0,
                                op0=mybir.AluOpType.add,
                                op1=mybir.AluOpType.max,
                            )
                        else:
                            nc.scalar.activation(
                                act_out[o][:, b * 512 : (b + 1) * 512],
                                pss_b[b][:],
                                func,
                                bias=bias[:, o : o + 1],
                            )
            else:
                for o in range(NO):
                    for b in range(NB):
                        ps = pss.tile(
                            [128, 512], F32, name=f"{pname}_{o}_{b}_u{u}", tag="ps"
                        )
                        for i in range(len(K)):
                            nc.tensor.matmul(
                                ps[:],
                                W[i][:, o * 128 : (o + 1) * 128],
                                rhs[i][:, b * 512 : (b + 1) * 512],
                                start=(i == 0),
                                stop=(i == len(K) - 1),
                            )
                        nc.scalar.activation(
                            act_out[o][:, b * 512 : (b + 1) * 512],
                            ps[:],
                            func,
                            bias=bias[:, o : o + 1],
                        )

        # ---------- per-sample compute (PE + ACT evac) ----------
        def emit_compute(s, u, P):
            act0 = [
                acts.tile([128, B], F16, name=f"act0_{o}_u{u}", tag=f"act0_{o}")
                for o in range(NO)
            ]
            emit_layer(
                u, "ps0", P["W0"], xT, K0, act0, AF.Relu, P["b0"], F16
            )

            act1 = [
                acts.tile([128, B], F16, name=f"act1_{o}_u{u}", tag=f"act1_{o}")
                for o in range(NO)
            ]
            emit_layer(
                u, "ps1", P["W1"], act0, K1, act1, AF.Relu, P["b1"], F16
            )

            out_t = outp.tile([16, B], F32, name=f"out_u{u}", tag="out")
            for b in range(NB):
                ps = pss.tile([128, 512], F32, name=f"psl_{b}_u{u}", tag="ps")
                for i in range(len(K1)):
                    nc.tensor.matmul(
                        ps[:D_OUT, :],
                        P["Wl"][i][:, :],
                        act1[i][:, b * 512 : (b + 1) * 512],
                        start=(i == 0),
                        stop=(i == len(K1) - 1),
                    )
                nc.vector.tensor_scalar_add(
                    out_t[:D_OUT, b * 512 : (b + 1) * 512],
                    ps[:D_OUT, :],
                    P["bl"][:, 0:1],
                )
            nc.sync.dma_start(out_d[s, :, :], out_t[:D_OUT, :])

        # software-pipeline: emit sample s+1's prep before sample s's compute
        # so the DVE queue stays ahead of the PE queue. `repeat` re-runs the
        # whole per-sample sequence (timing-only variants). Sample 0's prep
        # was emitted inside the setup interleave above.
        seq = [s for _ in range(repeat) for s in range(SPC)]
        assert seq[0] == 0
        preps = dict(W0=W0_0, W1=W1_0, Wl=Wl_0, b0=b0_0, b1=b1_0, bl=bl_0)
        for u, s in enumerate(seq):
            nxt = emit_prep(seq[u + 1], u + 1) if u + 1 < len(seq) else None
            emit_compute(s, u, preps)
            preps = nxt

    nc.compile()
    return nc


def _f16(a):
    return np.ascontiguousarray(np.asarray(a), dtype=np.float16)


def _bias_128(b):
    # (1024,) -> (128, 8): feature o = t*128 + p lands at [p, t]
    return np.ascontiguousarray(np.asarray(b, np.float32).reshape(NO, 128).T)


def _shard_inputs(inputs):
    x = np.asarray(inputs["x"])
    xT = _f16(x.T)
    shared = dict(
        xT=xT,
        wm0=_f16(inputs["wm0"]),
        wv0=_f16(inputs["wv0"]),
        wm1=_f16(inputs["wm1"]),
        wv1=_f16(inputs["wv1"]),
        wlm=_f16(inputs["wlm"]),
        wlv=_f16(inputs["wlv"]),
        bm0=_bias_128(inputs["bm0"]),
        bv0=_bias_128(inputs["bv0"]),
        bm1=_bias_128(inputs["bm1"]),
        bv1=_bias_128(inputs["bv1"]),
        blm=np.ascontiguousarray(
            np.asarray(inputs["blm"], np.float32).reshape(D_OUT, 1)
        ),
        blv=np.ascontiguousarray(
            np.asarray(inputs["blv"], np.float32).reshape(D_OUT, 1)
        ),
    )
    we0 = np.asarray(inputs["we0"])
    we1 = np.asarray(inputs["we1"])
    wel = np.asarray(inputs["wel"])
    be0 = np.asarray(inputs["be0"]).reshape(S, D_H)
    be1 = np.asarray(inputs["be1"]).reshape(S, D_H)
    bel = np.asarray(inputs["bel"]).reshape(S, D_OUT)

    in_maps = []
    for c in range(NCORES):
        sl = slice(c * SPC, (c + 1) * SPC)
        m = dict(shared)
        m["we0"] = _f16(we0[sl])
        m["we1"] = _f16(we1[sl])
        m["wel"] = _f16(wel[sl])
        m["be0"] = np.ascontiguousarray(
            be0[sl].astype(np.float32).reshape(SPC, NO, 128).transpose(0, 2, 1)
        )
        m["be1"] = np.ascontiguousarray(
            be1[sl].astype(np.float32).reshape(SPC, NO, 128).transpose(0, 2, 1)
        )
        m["bel"] = np.ascontiguousarray(
            bel[sl].astype(np.float32).reshape(SPC, D_OUT, 1)
        )
        in_maps.append(m)
    return in_maps


def _get_program(repeat=1, pair_b=True, evac_split=True, acts_bufs=2):
    key = (repeat, pair_b, evac_split, acts_bufs)
    if key not in _CACHE:
        _CACHE[key] = _build_program(repeat, pair_b, evac_split, acts_bufs)
    return _CACHE[key]


def run(inputs, trace=False, repeat=1, pair_b=True, evac_split=True, acts_bufs=2, **kw):
    nc = _get_program(repeat, pair_b, evac_split, acts_bufs)
    in_maps = _shard_inputs(inputs)
    try:
        res = run_bass_kernel_spmd(
            nc, in_maps, core_ids=list(range(NCORES)), trace=trace, **kw
        )
    except ModuleNotFoundError:
        # this container lacks the axon NTFF profile hook; rerun untraced
        os.environ["BASS_NEVER_TRACE"] = "1"
        res = run_bass_kernel_spmd(
            nc, in_maps, core_ids=list(range(NCORES)), trace=False, **kw
        )
    # per-core out: (SPC, 10, B) -> full (S, B, 10)
    parts = [
        np.transpose(res.results[c]["out"], (0, 2, 1)) for c in range(NCORES)
    ]
    full = np.concatenate(parts, axis=0).astype(np.float32)
    return full, res


def kernel(**inputs) -> np.ndarray:
    out, _ = run(inputs, trace=False)
    return out
